# revision 1
# baseline (speedup 1.0000x reference)
"""LayerNorm-GRU (nn_Encoder_Base) Trainium2 Bass kernel.

Contract: kernel(**inputs) takes FULL inputs (x [256,128,1024], W/U [1024,3072],
b [3072], gammas/betas [2,3072]) and returns the full output [256,128,1024].

Strategy: data-parallel over batch across 8 NeuronCores (32 rows/core, no
collectives).  Each core:
  Phase A: S1' = a*LN0(x@W+b)+c for all timesteps (M=128 matmul tiles), DRAM.
  Phase B: 128 sequential GRU steps; recurrent matmuls keep the batch as the
  stationary operand (lhsT = h^T chunks [128,32]); LN means come from an extra
  matmul column (host-appended column sums of U); sum-of-squares via ScalarE
  Square with accumulate; PE transposes maintain h^T / (r*h)^T.
"""

import numpy as np

_B, _T, _H = 256, 128, 1024
_ZR = 2 * _H          # 2048
_IN3 = 3 * _H         # 3072
_NCORES = 8
_BL = _B // _NCORES   # 32
_EPS = 1e-5

_CACHE = {}


def _build(affine, has_bias, n_steps):
    from concourse import bacc
    import concourse.tile as tile
    import concourse.mybir as mybir
    from concourse.masks import make_identity

    from contextlib import ExitStack

    f32 = mybir.dt.float32
    Alu = mybir.AluOpType
    Act = mybir.ActivationFunctionType

    nc = bacc.Bacc("TRN2", target_bir_lowering=False, debug=False,
                   enable_asserts=False, num_devices=_NCORES)

    x_d = nc.dram_tensor("x", [_BL, n_steps, _H], f32, kind="ExternalInput")
    w_d = nc.dram_tensor("w", [_H, _IN3], f32, kind="ExternalInput")
    u_d = nc.dram_tensor("u", [_H, _IN3 + 2], f32, kind="ExternalInput")
    o_d = nc.dram_tensor("o", [_BL, n_steps, _H], f32, kind="ExternalOutput")
    s1_d = nc.dram_tensor("s1", [_BL, n_steps, _IN3], f32)
    if has_bias:
        bias_d = nc.dram_tensor("bias", [_IN3], f32, kind="ExternalInput")
    if affine:
        # pa_scale/pa_bias: per-column affine applied to phase-A LN output
        #   (zr part already folded with 0.2 / +0.5 / beta1_zr on host)
        # g1: per-column scale for phase-B LN outputs (zr part pre-scaled 0.2)
        pas_d = nc.dram_tensor("pas", [_IN3], f32, kind="ExternalInput")
        pab_d = nc.dram_tensor("pab", [_IN3], f32, kind="ExternalInput")
        g1_d = nc.dram_tensor("g1", [_IN3], f32, kind="ExternalInput")

    def bcast(vec_ap, p=128):
        # broadcast a [N] dram vector across p partitions -> [p, N] AP
        import concourse.bass as bass
        return bass.AP(tensor=vec_ap.tensor, offset=vec_ap.offset,
                       ap=[[0, p]] + vec_ap.ap)

    with tile.TileContext(nc) as tc, ExitStack() as stack:
        persist = stack.enter_context(tc.tile_pool(name="persist", bufs=1))
        ident = persist.tile([128, 128], f32, tag="ident")
        make_identity(nc, ident)
        eps_sb = persist.tile([128, 1], f32, tag="eps_sb")
        nc.vector.memset(eps_sb, _EPS)

        wt_pool = stack.enter_context(tc.tile_pool(name="wt", bufs=1))
        w_sb = wt_pool.tile([128, 8, _IN3 + 2], f32, tag="wt")
        nc.sync.dma_start(out=w_sb[:, :, :_IN3],
                          in_=w_d.ap().rearrange("(k p) n -> p k n", p=128))

        consts = stack.enter_context(tc.tile_pool(name="consts", bufs=1))
        if has_bias:
            b_bc = consts.tile([128, _IN3], f32)
            nc.sync.dma_start(out=b_bc, in_=bcast(bias_d.ap()))
        if affine:
            pas_bc = consts.tile([128, _IN3], f32)
            nc.sync.dma_start(out=pas_bc, in_=bcast(pas_d.ap()))
            pab_bc = consts.tile([128, _IN3], f32)
            nc.sync.dma_start(out=pab_bc, in_=bcast(pab_d.ap()))
            g1_bc = consts.tile([32, _IN3], f32)
            nc.sync.dma_start(out=g1_bc, in_=bcast(g1_d.ap(), p=32))

        # ---------------- Phase A ----------------
        with tc.tile_pool(name="pa", bufs=2) as pa, \
             tc.tile_pool(name="pa_sm", bufs=2) as sm, \
             tc.tile_pool(name="pa_ps", bufs=1, space="PSUM") as pps:
            x_flat = x_d.ap().rearrange("b t h -> (b t) h")
            s1_flat = s1_d.ap().rearrange("b t n -> (b t) n")
            n_rows = _BL * n_steps
            assert n_rows % 128 == 0
            for bi in range(n_rows // 128):
                x_sb = pa.tile([128, _H], f32, tag="x", bufs=3)
                nc.sync.dma_start(out=x_sb, in_=x_flat[bi * 128:(bi + 1) * 128])
                psT = pps.tile([128, 8, 128], f32, tag="psT")
                for k in range(8):
                    nc.tensor.transpose(psT[:, k], x_sb[:, k * 128:(k + 1) * 128],
                                        ident)
                xT = pa.tile([128, 8, 128], f32, tag="xT")
                nc.vector.tensor_copy(out=xT, in_=psT)

                ps1 = pps.tile([128, _IN3], f32, tag="ps1")
                for k in range(8):
                    for n in range(6):
                        nc.tensor.matmul(
                            ps1[:, n * 512:(n + 1) * 512], xT[:, k],
                            w_sb[:, k, n * 512:(n + 1) * 512],
                            start=(k == 0), stop=(k == 7))

                if has_bias:
                    v = pa.tile([128, _IN3], f32, tag="v")
                    nc.vector.tensor_add(v, ps1, b_bc)
                else:
                    v = ps1

                stats = sm.tile([128, 6, 6], f32, tag="st")
                for g in range(6):
                    nc.vector.bn_stats(out=stats[:, g],
                                       in_=v[:, g * 512:(g + 1) * 512])
                mv = sm.tile([128, 2], f32, tag="mv")
                nc.vector.bn_aggr(out=mv, in_=stats)

                std = sm.tile([128, 1], f32, tag="std")
                nc.scalar.activation(out=std, in_=mv[:, 1:2], func=Act.Sqrt,
                                     bias=eps_sb)
                nc.vector.tensor_scalar_add(std, std, _EPS)
                rinv = sm.tile([128, 1], f32, tag="rinv")
                nc.vector.reciprocal(rinv, std)

                s1o = pa.tile([128, _IN3], f32, tag="s1o")
                if affine:
                    # raw LN, then per-column affine
                    nmr = sm.tile([128, 1], f32, tag="nmr")
                    nc.vector.tensor_mul(nmr, mv[:, 0:1], rinv)
                    nc.vector.tensor_scalar_mul(nmr, nmr, -1.0)
                    nc.scalar.activation(out=s1o, in_=v, func=Act.Identity,
                                         scale=rinv, bias=nmr)
                    nc.vector.tensor_mul(s1o, s1o, pas_bc)
                    nc.vector.tensor_add(s1o, s1o, pab_bc)
                else:
                    rinv02 = sm.tile([128, 1], f32, tag="rinv02")
                    nc.vector.tensor_scalar_mul(rinv02, rinv, 0.2)
                    nmr02 = sm.tile([128, 1], f32, tag="nmr02")
                    nc.vector.tensor_mul(nmr02, mv[:, 0:1], rinv02)
                    nc.vector.tensor_scalar(nmr02, nmr02, -1.0, 0.5,
                                            Alu.mult, Alu.add)
                    nc.scalar.activation(out=s1o[:, :_ZR], in_=v[:, :_ZR],
                                         func=Act.Identity, scale=rinv02,
                                         bias=nmr02)
                    nmr_c = sm.tile([128, 1], f32, tag="nmrc")
                    nc.vector.tensor_mul(nmr_c, mv[:, 0:1], rinv)
                    nc.vector.tensor_scalar_mul(nmr_c, nmr_c, -1.0)
                    nc.scalar.activation(out=s1o[:, _ZR:], in_=v[:, _ZR:],
                                         func=Act.Identity, scale=rinv,
                                         bias=nmr_c)
                nc.sync.dma_start(out=s1_flat[bi * 128:(bi + 1) * 128], in_=s1o)

        # ---------------- Phase B ----------------
        u_sb = w_sb  # same slot, reloaded with U (Tile serializes via WAR)
        nc.sync.dma_start(out=u_sb,
                          in_=u_d.ap().rearrange("(k p) n -> p k n", p=128))

        h_sb = persist.tile([32, _H], f32, tag="h_sb")
        nc.vector.memzero(h_sb)
        hT_sb = persist.tile([128, 8, 32], f32, tag="hT_sb")
        nc.vector.memzero(hT_sb)
        id32 = ident[:32, :32]

        with tc.tile_pool(name="pb", bufs=2) as pb, \
             tc.tile_pool(name="pb_sm", bufs=2) as sm, \
             tc.tile_pool(name="mm_ps", bufs=1, space="PSUM") as mm_ps, \
             tc.tile_pool(name="t_ps", bufs=1, space="PSUM") as t_ps:
            for t in range(n_steps):
                s1t = pb.tile([32, _IN3], f32, tag="s1t", bufs=3)
                nc.sync.dma_start(out=s1t, in_=s1_d.ap()[:, t, :])

                ps_zr = mm_ps.tile([32, _ZR], f32, tag="zr")
                ps_sums = mm_ps.tile([32, 2], f32, tag="sums")
                for k in range(8):
                    for n in range(4):
                        nc.tensor.matmul(
                            ps_zr[:, n * 512:(n + 1) * 512], hT_sb[:, k],
                            w_sb[:, k, n * 512:(n + 1) * 512],
                            start=(k == 0), stop=(k == 7))
                    nc.tensor.matmul(ps_sums[:, 0:1], hT_sb[:, k],
                                     w_sb[:, k, _IN3:_IN3 + 1],
                                     start=(k == 0), stop=(k == 7))

                # LN stats for zr (N=2048)
                ssq = sm.tile([32, 1], f32, tag="ssq")
                s2a = pb.tile([32, _ZR], f32, tag="s2a")
                nc.scalar.activation(out=s2a, in_=ps_zr, func=Act.Square,
                                     accum_out=ssq)
                m = sm.tile([32, 1], f32, tag="m")
                nc.vector.tensor_scalar_mul(m, ps_sums[:, 0:1], 1.0 / _ZR)
                var = sm.tile([32, 1], f32, tag="var")
                nc.vector.tensor_mul(var, m, m)
                nc.vector.scalar_tensor_tensor(
                    out=var, in0=ssq, scalar=1.0 / _ZR, in1=var,
                    op0=Alu.mult, op1=Alu.subtract)
                std = sm.tile([32, 1], f32, tag="std")
                nc.scalar.activation(out=std, in_=var, func=Act.Sqrt,
                                     bias=eps_sb[:32])
                nc.vector.tensor_scalar_add(std, std, _EPS)
                rinv = sm.tile([32, 1], f32, tag="rinv")
                nc.vector.reciprocal(rinv, std)

                if affine:
                    nmr = sm.tile([32, 1], f32, tag="nmr")
                    nc.vector.tensor_mul(nmr, m, rinv)
                    nc.vector.tensor_scalar_mul(nmr, nmr, -1.0)
                    nc.scalar.activation(out=s2a, in_=ps_zr, func=Act.Identity,
                                         scale=rinv, bias=nmr)
                    nc.vector.tensor_mul(s2a, s2a, g1_bc[:, :_ZR])
                else:
                    rinv02 = sm.tile([32, 1], f32, tag="rinv02")
                    nc.vector.tensor_scalar_mul(rinv02, rinv, 0.2)
                    nmr02 = sm.tile([32, 1], f32, tag="nmr02")
                    nc.vector.tensor_mul(nmr02, m, rinv02)
                    nc.vector.tensor_scalar_mul(nmr02, nmr02, -1.0)
                    nc.scalar.activation(out=s2a, in_=ps_zr, func=Act.Identity,
                                         scale=rinv02, bias=nmr02)

                # s = clip(s1'_zr + s2a, 0, 1)   (0.5 folded into s1')
                s_sb = pb.tile([32, _ZR], f32, tag="s")
                nc.vector.tensor_add(s_sb, s2a, s1t[:, :_ZR])
                nc.vector.tensor_scalar(s_sb, s_sb, 1.0, 0.0, Alu.min, Alu.max)

                # rh = r * h ; transpose
                rh = pb.tile([32, _H], f32, tag="rh")
                nc.vector.tensor_mul(rh, s_sb[:, _H:], h_sb)
                psT = t_ps.tile([128, 8, 32], f32, tag="T")
                for k in range(8):
                    nc.tensor.transpose(psT[:, k], rh[:, k * 128:(k + 1) * 128],
                                        id32)
                rhT = pb.tile([128, 8, 32], f32, tag="rhT")
                nc.vector.tensor_copy(out=rhT, in_=psT)

                ps_c = mm_ps.tile([32, _H], f32, tag="c")
                for k in range(8):
                    for n in range(2):
                        nc.tensor.matmul(
                            ps_c[:, n * 512:(n + 1) * 512], rhT[:, k],
                            w_sb[:, k, _ZR + n * 512:_ZR + (n + 1) * 512],
                            start=(k == 0), stop=(k == 7))
                    nc.tensor.matmul(ps_sums[:, 1:2], rhT[:, k],
                                     w_sb[:, k, _IN3 + 1:_IN3 + 2],
                                     start=(k == 0), stop=(k == 7))

                # LN stats for c (N=1024)
                ssq_c = sm.tile([32, 1], f32, tag="ssqc")
                cand = pb.tile([32, _H], f32, tag="cand")
                nc.scalar.activation(out=cand, in_=ps_c, func=Act.Square,
                                     accum_out=ssq_c)
                m_c = sm.tile([32, 1], f32, tag="mc")
                nc.vector.tensor_scalar_mul(m_c, ps_sums[:, 1:2], 1.0 / _H)
                var_c = sm.tile([32, 1], f32, tag="varc")
                nc.vector.tensor_mul(var_c, m_c, m_c)
                nc.vector.scalar_tensor_tensor(
                    out=var_c, in0=ssq_c, scalar=1.0 / _H, in1=var_c,
                    op0=Alu.mult, op1=Alu.subtract)
                std_c = sm.tile([32, 1], f32, tag="stdc")
                nc.scalar.activation(out=std_c, in_=var_c, func=Act.Sqrt,
                                     bias=eps_sb[:32])
                nc.vector.tensor_scalar_add(std_c, std_c, _EPS)
                rinv_c = sm.tile([32, 1], f32, tag="rinvc")
                nc.vector.reciprocal(rinv_c, std_c)
                nmr_c = sm.tile([32, 1], f32, tag="nmrc")
                nc.vector.tensor_mul(nmr_c, m_c, rinv_c)
                nc.vector.tensor_scalar_mul(nmr_c, nmr_c, -1.0)

                if affine:
                    lnc = pb.tile([32, _H], f32, tag="lnc")
                    nc.scalar.activation(out=lnc, in_=ps_c, func=Act.Identity,
                                         scale=rinv_c, bias=nmr_c)
                    nc.vector.tensor_mul(lnc, lnc, g1_bc[:, _ZR:])
                    nc.vector.tensor_add(cand, lnc, s1t[:, _ZR:])
                else:
                    s1c = pb.tile([32, _H], f32, tag="s1c")
                    nc.vector.tensor_scalar_add(s1c, s1t[:, _ZR:], nmr_c)
                    nc.vector.scalar_tensor_tensor(
                        out=cand, in0=ps_c, scalar=rinv_c, in1=s1c,
                        op0=Alu.mult, op1=Alu.add)
                nc.scalar.activation(out=cand, in_=cand, func=Act.Tanh)

                # h_new = cand + z*(h - cand)
                d = pb.tile([32, _H], f32, tag="d")
                nc.vector.tensor_sub(d, h_sb, cand)
                nc.vector.tensor_mul(d, d, s_sb[:, :_H])
                nc.vector.tensor_add(h_sb, d, cand)

                for k in range(8):
                    nc.tensor.transpose(psT[:, k],
                                        h_sb[:, k * 128:(k + 1) * 128], id32)
                nc.vector.tensor_copy(out=hT_sb, in_=psT)
                nc.sync.dma_start(out=o_d.ap()[:, t, :], in_=h_sb)

    nc.compile()
    return nc


def _get_nc(affine, has_bias, n_steps):
    key = (affine, has_bias, n_steps)
    if key not in _CACHE:
        _CACHE[key] = _build(affine, has_bias, n_steps)
    return _CACHE[key]


LAST_RESULTS = None


def kernel(x, W, U, b, gammas, betas, n_steps=_T, trace=False):
    global LAST_RESULTS
    from concourse.bass_utils import run_bass_kernel_spmd

    x = np.ascontiguousarray(np.asarray(x, dtype=np.float32))[:, :n_steps]
    W = np.asarray(W, dtype=np.float32)
    U = np.asarray(U, dtype=np.float32)
    b = np.asarray(b, dtype=np.float32)
    gammas = np.asarray(gammas, dtype=np.float32)
    betas = np.asarray(betas, dtype=np.float32)

    has_bias = bool(np.any(b != 0.0))
    affine = bool(np.any(gammas != 1.0) or np.any(betas != 0.0))

    u_ext = np.concatenate(
        [U, U[:, :_ZR].sum(1, keepdims=True, dtype=np.float64).astype(np.float32),
         U[:, _ZR:].sum(1, keepdims=True, dtype=np.float64).astype(np.float32)],
        axis=1)

    nc = _get_nc(affine, has_bias, n_steps)

    in_maps = []
    for c in range(_NCORES):
        m = {"x": np.ascontiguousarray(x[c * _BL:(c + 1) * _BL]),
             "w": W, "u": u_ext}
        if has_bias:
            m["bias"] = b
        if affine:
            g0, b0 = gammas[0], betas[0]
            g1, b1 = gammas[1], betas[1]
            pas = np.concatenate([0.2 * g0[:_ZR], g0[_ZR:]])
            pab = np.concatenate(
                [0.2 * b0[:_ZR] + 0.5 + 0.2 * b1[:_ZR], b0[_ZR:] + b1[_ZR:]])
            g1v = np.concatenate([0.2 * g1[:_ZR], g1[_ZR:]])
            m["pas"] = pas.astype(np.float32)
            m["pab"] = pab.astype(np.float32)
            m["g1"] = g1v.astype(np.float32)
        in_maps.append(m)

    res = run_bass_kernel_spmd(nc, in_maps, list(range(_NCORES)), trace=trace)
    LAST_RESULTS = res
    out = np.concatenate([res.results[c]["o"] for c in range(_NCORES)], axis=0)
    return out

